# revision 43
# baseline (speedup 1.0000x reference)
"""Bahdanau attention on 8 Trainium2 NeuronCores.

Full inputs in, full outputs out. Batch (B=32) is sharded 4-per-core
(data parallel); all weights are replicated. Per core, for each of its
4 batches:

    e_proj[s, k]  = sum_h enc[b, s, h] * U_w[k, h]          (bf16 PE matmul,
                     s on partitions, k on the free axis)
    t[s, k]       = tanh(e_proj[s, k] + h_proj[b, k] + W_b[k] + U_b[k])
                     (DVE add of the broadcast row bias, then ACT tanh)
    scores[s]     = sum_k V[k] * t[s, k]                     (DVE fused
                     multiply + free-axis reduce — keeps the PE free)
    a[s]          = exp(scores[s] + V_b)                     (no max needed:
                     |scores| <= ||V||_1 + |V_b| ~ 26, exp fits fp32 easily)
    attention     = a / sum(a)        (sum via tiny ones-matmuls in PSUM)
    context[h]    = sum_s a[s] * enc[b, s, h] / sum(a)       (bf16 PE matmul)

The h-contraction needs encoder tiles with h on partitions while the
s-contraction needs s on partitions, so the host passes the encoder twice
(bf16 transposed + bf16 natural); 32 MB/core of DMA under ~280 us of PE work.
Scores emerge as 128-deep columns, which is exactly the lhsT layout the
context matmul needs — no on-chip transposes anywhere.
"""

from contextlib import ExitStack

import numpy as np
import ml_dtypes

import concourse.bass as bass  # noqa: F401
import concourse.mybir as mybir
from concourse import tile, bacc
from concourse.bass_utils import run_bass_kernel_spmd

BF16 = mybir.dt.bfloat16
F32 = mybir.dt.float32
AF = mybir.ActivationFunctionType

B, S, H = 32, 2048, 1024
NCORES = 8
BL = B // NCORES      # 4 batches per core
P = 128
KC = H // P           # 8 contraction chunks
SBLK = 4              # s-blocks per batch (DMA granularity)
SW = S // SBLK        # 512 rows per s-block
CH = SW // P          # 4 s-chunks of 128 per s-block
NQ = S // P           # 16 s-chunks per batch


def build_program(v_b: float) -> bacc.Bacc:
    nc = bacc.Bacc("TRN2", target_bir_lowering=False, debug=False, num_devices=NCORES)

    encT_d = nc.dram_tensor("encT", [BL, H, S], BF16, kind="ExternalInput")
    encN_d = nc.dram_tensor("encN", [BL, S, H], BF16, kind="ExternalInput")
    uwT_d = nc.dram_tensor("uwT", [H, H], BF16, kind="ExternalInput")
    wwT_d = nc.dram_tensor("wwT", [H, H], BF16, kind="ExternalInput")
    hidT_d = nc.dram_tensor("hidT", [H, BL], BF16, kind="ExternalInput")
    wub4_d = nc.dram_tensor("wub4", [BL, H], F32, kind="ExternalInput")
    v128_d = nc.dram_tensor("v128", [P, H], BF16, kind="ExternalInput")
    ctx_d = nc.dram_tensor("ctx_out", [BL, H], F32, kind="ExternalOutput")
    # column-major: attn_out[b, p, q] = attention[b, q*128 + p] (host reshapes)
    attn_d = nc.dram_tensor("attn_out", [BL, P, NQ], F32, kind="ExternalOutput")
    hrow_dram = nc.dram_tensor("hrow_scratch", [BL, H], F32)
    rec_dram = nc.dram_tensor("rec_scratch", [BL, 1], F32)

    with tile.TileContext(nc) as tc, ExitStack() as stack:
        const = stack.enter_context(tc.tile_pool(name="const", bufs=1))

        # sync (SP) HWDGE ring: the big bf16 operands the main loop needs first
        uw_t = const.tile([P, KC, H], BF16, name="uw_t")
        nc.sync.dma_start(
            out=uw_t[:], in_=uwT_d.ap().rearrange("(kc p) n -> p kc n", p=P)
        )
        # scalar (ACT) HWDGE ring: everything h_proj needs, in parallel
        hid_t = const.tile([P, KC, BL], BF16, name="hid_t")
        nc.scalar.dma_start(
            out=hid_t[:], in_=hidT_d.ap().rearrange("(kc p) b -> p kc b", p=P)
        )
        wub4_t = const.tile([BL, H], F32, name="wub4_t")
        v128_t = const.tile([P, H], BF16, name="v128_t")
        one_t = const.tile([P, 1], BF16, name="one_t")
        nc.vector.memset(one_t[:], 1.0)
        vb_t = const.tile([P, 1], F32, name="vb_t")
        nc.vector.memset(vb_t[:], v_b)
        # hbrow[b, k] = h_proj[b, k] + W_b[k] + U_b[k]; hb128[.] broadcasts
        # the current batch's row across all 128 partitions (gpsimd).
        hbrow_t = const.tile([BL, H], F32, name="hbrow_t")

        # ---- phase 0: h_proj rows. lhsT = hidden columns (LDWEIGHTS is only
        # 4 cols) so this is 16 wide fp32 matmuls, nothing else.
        def emit_hproj():
            with (
                tc.tile_pool(name="wpool", bufs=1) as wpool,
                tc.tile_pool(name="hrow_psum", bufs=1, space="PSUM") as hrow_psum,
            ):
                ww_src = wwT_d.ap().rearrange("(kc p) n -> p kc n", p=P)
                ww_ts = []
                for kc in range(KC):
                    ww_kc = wpool.tile([P, H], BF16, name=f"ww_{kc}")
                    nc.scalar.dma_start(out=ww_kc[:], in_=ww_src[:, kc, :])
                    ww_ts.append(ww_kc)
                nc.scalar.dma_start(out=wub4_t[:], in_=wub4_d.ap())
                nc.scalar.dma_start(out=v128_t[:], in_=v128_d.ap())
                hrow_ps = [
                    hrow_psum.tile([BL, 512], F32, name=f"hrow_ps{h}") for h in range(2)
                ]
                for kc in range(KC):
                    for h in range(2):
                        nc.tensor.matmul(
                            hrow_ps[h][:],
                            hid_t[:, kc, :],
                            ww_ts[kc][:, h * 512 : (h + 1) * 512],
                            start=(kc == 0),
                            stop=(kc == KC - 1),
                        )
                for h in range(2):
                    nc.vector.tensor_add(
                        hbrow_t[:, h * 512 : (h + 1) * 512],
                        hrow_ps[h][:],
                        wub4_t[:, h * 512 : (h + 1) * 512],
                    )

        # ---- main pools (ep_psum first: disjoint banks from phase-0 pools)
        encTp = stack.enter_context(tc.tile_pool(name="encTp", bufs=3))
        encNp = stack.enter_context(tc.tile_pool(name="encNp", bufs=3))
        esp = stack.enter_context(tc.tile_pool(name="esp", bufs=3))
        tanhp = stack.enter_context(tc.tile_pool(name="tanhp", bufs=3))
        scrp = stack.enter_context(tc.tile_pool(name="scrp", bufs=2))
        accp = stack.enter_context(tc.tile_pool(name="accp", bufs=3))
        acolp = stack.enter_context(tc.tile_pool(name="acolp", bufs=2))
        hbp = stack.enter_context(tc.tile_pool(name="hbp", bufs=2))
        outp = stack.enter_context(tc.tile_pool(name="outp", bufs=2))
        ep_psum = stack.enter_context(tc.tile_pool(name="ep_psum", bufs=2, space="PSUM"))
        late = {}

        def late_pools():
            late["ctx"] = stack.enter_context(
                tc.tile_pool(name="ctx_psum", bufs=1, space="PSUM")
            )
            late["den"] = stack.enter_context(
                tc.tile_pool(name="den_psum", bufs=1, space="PSUM")
            )

        pending = []  # deferred PE work (ctx/den matmuls), staggered 2 chunks

        def drain_pending(keep=2):
            while len(pending) > keep:
                pending.pop(0)()

        emit_hproj()
        late_pools()

        for b in range(BL):
            hb128_t = hbp.tile([P, H], F32, name="hb128_t")
            if b == 0:
                nc.gpsimd.partition_broadcast(
                    hb128_t[:], hbrow_t[0:1, :], channels=P
                )
            else:
                # replicate this batch's bias row to all partitions via DRAM
                nc.sync.dma_start(
                    out=hrow_dram.ap()[b : b + 1, :], in_=hbrow_t[b : b + 1, :]
                )
                nc.sync.dma_start(
                    out=hb128_t[:], in_=hrow_dram.ap()[b].partition_broadcast(P)
                )
            acol_t = acolp.tile([P, NQ], BF16, name="acol_t")
            cps = {}

            def make_ctx(q, encN_t, c):
                def emit_ctx():
                    if not cps:
                        cps["c0"] = late["ctx"].tile([1, 512], F32, name="ctx_ps0")
                        cps["c1"] = late["ctx"].tile([1, 512], F32, name="ctx_ps1")
                        cps["d"] = late["den"].tile([1, 1], F32, name="den_ps")
                    st = q == 0
                    sp = q == NQ - 1
                    nc.tensor.matmul(
                        cps["c0"][:], acol_t[:, q : q + 1], encN_t[:, c, 0:512],
                        start=st, stop=sp, skip_group_check=True,
                    )
                    nc.tensor.matmul(
                        cps["c1"][:], acol_t[:, q : q + 1], encN_t[:, c, 512:1024],
                        start=st, stop=sp, skip_group_check=True,
                    )
                    nc.tensor.matmul(
                        cps["d"][:], acol_t[:, q : q + 1], one_t[:],
                        start=st, stop=sp, skip_group_check=True,
                    )

                return emit_ctx

            for sb in range(SBLK):
                encT_t = encTp.tile([P, KC, SW], BF16, name="encT_t")
                nc.sync.dma_start(
                    out=encT_t[:],
                    in_=encT_d.ap()[b].rearrange("(kc p) s -> p kc s", p=P)[
                        :, :, sb * SW : (sb + 1) * SW
                    ],
                )
                encN_t = encNp.tile([P, CH, H], BF16, name="encN_t")

                def emit_encN(encN_t=encN_t, b=b, sb=sb):
                    nc.sync.dma_start(
                        out=encN_t[:],
                        in_=encN_d.ap()[b].rearrange("(c p) h -> p c h", p=P)[
                            :, sb * CH : (sb + 1) * CH, :
                        ],
                    )

                # batch 0 block 0: issue encN after the first compute chunk so
                # the startup HBM ramp prioritizes uw/ww/encT (encN is first
                # read by the trailing context matmuls, ~10us later)
                encN_wait = b == 0 and sb == 0
                if not encN_wait:
                    emit_encN()

                for c in range(CH):
                    q = sb * CH + c
                    # e_proj for 128 s-rows: [128, 1024] over two psum banks
                    ep_ps = ep_psum.tile([P, H], F32, name="ep_ps")
                    for kc in range(KC):
                        for h in range(2):
                            nc.tensor.matmul(
                                ep_ps[:, h * 512 : (h + 1) * 512],
                                encT_t[:, kc, c * P : (c + 1) * P],
                                uw_t[:, kc, h * 512 : (h + 1) * 512],
                                start=(kc == 0),
                                stop=(kc == KC - 1),
                            )

                    def emit_chain(q, ep_ps):
                        # + row bias, tanh, V-weighted reduce -> scores column
                        es_t = esp.tile([P, H], F32, name="es_t")
                        nc.vector.tensor_add(es_t[:], ep_ps[:], hb128_t[:])
                        tanh_t = tanhp.tile([P, H], BF16, name="tanh_t")
                        nc.scalar.activation(tanh_t[:], es_t[:], AF.Tanh)
                        scr_t = scrp.tile([P, H], BF16, name="scr_t")
                        nc.vector.tensor_mul(scr_t[:], tanh_t[:], v128_t[:])
                        acc_t = accp.tile([P, 1], F32, name="acc_t")
                        nc.vector.reduce_sum(
                            acc_t[:], scr_t[:], axis=mybir.AxisListType.X
                        )
                        # a-column = exp(scores + V_b), already in lhsT layout
                        nc.scalar.activation(
                            acol_t[:, q : q + 1], acc_t[:], AF.Exp, bias=vb_t[:]
                        )

                    emit_chain(q, ep_ps)
                    if encN_wait:
                        emit_encN()
                        encN_wait = False
                    pending.append(make_ctx(q, encN_t, c))
                    drain_pending(keep=2)

            drain_pending(keep=0)  # batch boundary: finish ctx/den for b
            ctx_ps0, ctx_ps1, den_ps = cps["c0"], cps["c1"], cps["d"]

            den_t = outp.tile([1, 1], F32, name="den_t")
            rec_t = outp.tile([1, 1], F32, name="rec_t")
            nc.vector.tensor_copy(den_t[:], den_ps[:])
            nc.vector.reciprocal(rec_t[:], den_t[:])
            rec128_t = accp.tile([P, 1], F32, name="rec128_t")
            nc.gpsimd.partition_broadcast(rec128_t[:], rec_t[:], channels=P)

            acoln_t = outp.tile([P, NQ], F32, name="acoln_t")
            nc.vector.tensor_scalar_mul(acoln_t[:], acol_t[:], rec128_t[:])
            nc.sync.dma_start(out=attn_d.ap()[b], in_=acoln_t[:])

            cs_t = outp.tile([1, H], F32, name="cs_t")
            nc.vector.tensor_scalar_mul(cs_t[0:1, 0:512], ctx_ps0[:], rec_t[:])
            nc.vector.tensor_scalar_mul(cs_t[0:1, 512:1024], ctx_ps1[:], rec_t[:])
            nc.sync.dma_start(out=ctx_d.ap()[b : b + 1, :], in_=cs_t[:])

    nc.compile()
    return nc


def _prep_inputs(hidden, enc, W_w, W_b, U_w, U_b, V_w):
    bf16 = ml_dtypes.bfloat16
    uwT = np.ascontiguousarray(U_w.T).astype(bf16)
    wwT = np.ascontiguousarray(W_w.T).astype(bf16)
    wub4 = np.ascontiguousarray(
        np.broadcast_to((W_b + U_b)[None, :], (BL, H))
    ).astype(np.float32)
    v128 = np.ascontiguousarray(
        np.broadcast_to(V_w.reshape(1, H), (P, H))
    ).astype(bf16)

    in_maps = []
    for i in range(NCORES):
        sl = slice(i * BL, (i + 1) * BL)
        e = enc[sl]
        in_maps.append(
            {
                "encT": np.ascontiguousarray(e.transpose(0, 2, 1)).astype(bf16),
                "encN": e.astype(bf16),
                "uwT": uwT,
                "wwT": wwT,
                "hidT": np.ascontiguousarray(hidden[sl, 0, :].T).astype(bf16),
                "wub4": wub4,
                "v128": v128,
            }
        )
    return in_maps


def run(inputs: dict, trace: bool = False):
    """Build + run; returns ((context, attention), BassKernelResults)."""
    hidden = np.asarray(inputs["hidden"], dtype=np.float32)
    enc = np.asarray(inputs["encoder_output"], dtype=np.float32)
    W_w = np.asarray(inputs["W_w"], dtype=np.float32)
    W_b = np.asarray(inputs["W_b"], dtype=np.float32)
    U_w = np.asarray(inputs["U_w"], dtype=np.float32)
    U_b = np.asarray(inputs["U_b"], dtype=np.float32)
    V_w = np.asarray(inputs["V_w"], dtype=np.float32)
    V_b = np.asarray(inputs["V_b"], dtype=np.float32)

    nc = build_program(float(V_b.reshape(-1)[0]))
    in_maps = _prep_inputs(hidden, enc, W_w, W_b, U_w, U_b, V_w)
    try:
        res = run_bass_kernel_spmd(nc, in_maps, list(range(NCORES)), trace=trace)
    except Exception:
        # a previously wedged NeuronCore can fail the first execute; one
        # clean retry recovers it
        res = run_bass_kernel_spmd(nc, in_maps, list(range(NCORES)), trace=trace)

    ctx = np.concatenate(
        [np.asarray(res.results[i]["ctx_out"]) for i in range(NCORES)], axis=0
    ).astype(np.float32)[:, None, :]
    attn_col = np.concatenate(
        [np.asarray(res.results[i]["attn_out"]) for i in range(NCORES)], axis=0
    ).astype(np.float32)
    # attn_col[b, p, q] holds attention[b, q*128 + p]
    attn = attn_col.transpose(0, 2, 1).reshape(B, S)[:, None, :]
    return (ctx, attn), res


def kernel(**inputs) -> tuple:
    out, _ = run(inputs, trace=False)
    return out


# revision 44
# speedup vs baseline: 1.0055x; 1.0055x over previous
"""Bahdanau attention on 8 Trainium2 NeuronCores.

Full inputs in, full outputs out. Batch (B=32) is sharded 4-per-core
(data parallel); all weights are replicated. Per core, for each of its
4 batches:

    e_proj[s, k]  = sum_h enc[b, s, h] * U_w[k, h]          (bf16 PE matmul,
                     s on partitions, k on the free axis)
    t[s, k]       = tanh(e_proj[s, k] + h_proj[b, k] + W_b[k] + U_b[k])
                     (DVE add of the broadcast row bias, then ACT tanh)
    scores[s]     = sum_k V[k] * t[s, k]                     (DVE fused
                     multiply + free-axis reduce — keeps the PE free)
    a[s]          = exp(scores[s] + V_b)                     (no max needed:
                     |scores| <= ||V||_1 + |V_b| ~ 26, exp fits fp32 easily)
    attention     = a / sum(a)        (sum via tiny ones-matmuls in PSUM)
    context[h]    = sum_s a[s] * enc[b, s, h] / sum(a)       (bf16 PE matmul)

The h-contraction needs encoder tiles with h on partitions while the
s-contraction needs s on partitions, so the host passes the encoder twice
(bf16 transposed + bf16 natural); 32 MB/core of DMA under ~280 us of PE work.
Scores emerge as 128-deep columns, which is exactly the lhsT layout the
context matmul needs — no on-chip transposes anywhere.
"""

from contextlib import ExitStack

import numpy as np
import ml_dtypes

import concourse.bass as bass  # noqa: F401
import concourse.mybir as mybir
from concourse import tile, bacc
from concourse.bass_utils import run_bass_kernel_spmd

BF16 = mybir.dt.bfloat16
F32 = mybir.dt.float32
AF = mybir.ActivationFunctionType

B, S, H = 32, 2048, 1024
NCORES = 8
BL = B // NCORES      # 4 batches per core
P = 128
KC = H // P           # 8 contraction chunks
SBLK = 4              # s-blocks per batch (DMA granularity)
SW = S // SBLK        # 512 rows per s-block
CH = SW // P          # 4 s-chunks of 128 per s-block
NQ = S // P           # 16 s-chunks per batch


def build_program(v_b: float) -> bacc.Bacc:
    nc = bacc.Bacc("TRN2", target_bir_lowering=False, debug=False, num_devices=NCORES)

    encT_d = nc.dram_tensor("encT", [BL, H, S], BF16, kind="ExternalInput")
    encN_d = nc.dram_tensor("encN", [BL, S, H], BF16, kind="ExternalInput")
    uwT_d = nc.dram_tensor("uwT", [H, H], BF16, kind="ExternalInput")
    wwT_d = nc.dram_tensor("wwT", [H, H], BF16, kind="ExternalInput")
    hidT_d = nc.dram_tensor("hidT", [H, BL], BF16, kind="ExternalInput")
    wub4_d = nc.dram_tensor("wub4", [BL, H], F32, kind="ExternalInput")
    v128_d = nc.dram_tensor("v128", [P, H], BF16, kind="ExternalInput")
    ctx_d = nc.dram_tensor("ctx_out", [BL, H], F32, kind="ExternalOutput")
    # column-major: attn_out[b, p, q] = attention[b, q*128 + p] (host reshapes)
    attn_d = nc.dram_tensor("attn_out", [BL, P, NQ], F32, kind="ExternalOutput")
    hrow_dram = nc.dram_tensor("hrow_scratch", [BL, H], F32)
    rec_dram = nc.dram_tensor("rec_scratch", [BL, 1], F32)

    with tile.TileContext(nc) as tc, ExitStack() as stack:
        const = stack.enter_context(tc.tile_pool(name="const", bufs=1))

        # sync (SP) HWDGE ring: the big bf16 operands the main loop needs first
        uw_t = const.tile([P, KC, H], BF16, name="uw_t")
        nc.sync.dma_start(
            out=uw_t[:], in_=uwT_d.ap().rearrange("(kc p) n -> p kc n", p=P)
        )
        # scalar (ACT) HWDGE ring: everything h_proj needs, in parallel
        hid_t = const.tile([P, KC, BL], BF16, name="hid_t")
        nc.scalar.dma_start(
            out=hid_t[:], in_=hidT_d.ap().rearrange("(kc p) b -> p kc b", p=P)
        )
        wub4_t = const.tile([BL, H], F32, name="wub4_t")
        v128_t = const.tile([P, H], BF16, name="v128_t")
        one_t = const.tile([P, 1], BF16, name="one_t")
        nc.vector.memset(one_t[:], 1.0)
        vb_t = const.tile([P, 1], F32, name="vb_t")
        nc.vector.memset(vb_t[:], v_b)
        # hbrow[b, k] = h_proj[b, k] + W_b[k] + U_b[k]; hb128[.] broadcasts
        # the current batch's row across all 128 partitions (gpsimd).
        hbrow_t = const.tile([BL, H], F32, name="hbrow_t")

        # ---- phase 0: h_proj rows. lhsT = hidden columns (LDWEIGHTS is only
        # 4 cols) so this is 16 wide fp32 matmuls, nothing else.
        def emit_hproj():
            with (
                tc.tile_pool(name="wpool", bufs=1) as wpool,
                tc.tile_pool(name="hrow_psum", bufs=1, space="PSUM") as hrow_psum,
            ):
                ww_src = wwT_d.ap().rearrange("(kc p) n -> p kc n", p=P)
                ww_ts = []
                for kc in range(KC):
                    ww_kc = wpool.tile([P, H], BF16, name=f"ww_{kc}")
                    nc.scalar.dma_start(out=ww_kc[:], in_=ww_src[:, kc, :])
                    ww_ts.append(ww_kc)
                nc.scalar.dma_start(out=wub4_t[:], in_=wub4_d.ap())
                nc.scalar.dma_start(out=v128_t[:], in_=v128_d.ap())
                hrow_ps = [
                    hrow_psum.tile([BL, 512], F32, name=f"hrow_ps{h}") for h in range(2)
                ]
                for kc in range(KC):
                    for h in range(2):
                        nc.tensor.matmul(
                            hrow_ps[h][:],
                            hid_t[:, kc, :],
                            ww_ts[kc][:, h * 512 : (h + 1) * 512],
                            start=(kc == 0),
                            stop=(kc == KC - 1),
                        )
                for h in range(2):
                    nc.vector.tensor_add(
                        hbrow_t[:, h * 512 : (h + 1) * 512],
                        hrow_ps[h][:],
                        wub4_t[:, h * 512 : (h + 1) * 512],
                    )

        # ---- main pools (ep_psum first: disjoint banks from phase-0 pools)
        encTp = stack.enter_context(tc.tile_pool(name="encTp", bufs=3))
        encNp = stack.enter_context(tc.tile_pool(name="encNp", bufs=3))
        esp = stack.enter_context(tc.tile_pool(name="esp", bufs=3))
        tanhp = stack.enter_context(tc.tile_pool(name="tanhp", bufs=3))
        scrp = stack.enter_context(tc.tile_pool(name="scrp", bufs=2))
        accp = stack.enter_context(tc.tile_pool(name="accp", bufs=3))
        acolp = stack.enter_context(tc.tile_pool(name="acolp", bufs=2))
        hbp = stack.enter_context(tc.tile_pool(name="hbp", bufs=2))
        outp = stack.enter_context(tc.tile_pool(name="outp", bufs=2))
        ep_psum = stack.enter_context(tc.tile_pool(name="ep_psum", bufs=2, space="PSUM"))
        late = {}

        def late_pools():
            late["ctx"] = stack.enter_context(
                tc.tile_pool(name="ctx_psum", bufs=1, space="PSUM")
            )
            late["den"] = stack.enter_context(
                tc.tile_pool(name="den_psum", bufs=1, space="PSUM")
            )

        pending = []  # deferred PE work (ctx/den matmuls), staggered 2 chunks

        def drain_pending(keep=2):
            while len(pending) > keep:
                pending.pop(0)()

        emit_hproj()
        late_pools()

        for b in range(BL):
            hb128_t = hbp.tile([P, H], F32, name="hb128_t")
            if b == 0:
                nc.gpsimd.partition_broadcast(
                    hb128_t[:], hbrow_t[0:1, :], channels=P
                )
            else:
                # replicate this batch's bias row to all partitions via DRAM
                nc.sync.dma_start(
                    out=hrow_dram.ap()[b : b + 1, :], in_=hbrow_t[b : b + 1, :]
                )
                nc.sync.dma_start(
                    out=hb128_t[:], in_=hrow_dram.ap()[b].partition_broadcast(P)
                )
            acol_t = acolp.tile([P, NQ], BF16, name="acol_t")
            cps = {}

            def make_ctx(q, encN_t, c):
                def emit_ctx():
                    if not cps:
                        cps["c0"] = late["ctx"].tile([1, 512], F32, name="ctx_ps0")
                        cps["c1"] = late["ctx"].tile([1, 512], F32, name="ctx_ps1")
                        cps["d"] = late["den"].tile([1, 1], F32, name="den_ps")
                    st = q == 0
                    sp = q == NQ - 1
                    nc.tensor.matmul(
                        cps["c0"][:], acol_t[:, q : q + 1], encN_t[:, c, 0:512],
                        start=st, stop=sp, skip_group_check=True,
                    )
                    nc.tensor.matmul(
                        cps["c1"][:], acol_t[:, q : q + 1], encN_t[:, c, 512:1024],
                        start=st, stop=sp, skip_group_check=True,
                    )
                    nc.tensor.matmul(
                        cps["d"][:], acol_t[:, q : q + 1], one_t[:],
                        start=st, stop=sp, skip_group_check=True,
                    )

                return emit_ctx

            for sb in range(SBLK):
                encT_t = encTp.tile([P, KC, SW], BF16, name="encT_t")
                nc.sync.dma_start(
                    out=encT_t[:],
                    in_=encT_d.ap()[b].rearrange("(kc p) s -> p kc s", p=P)[
                        :, :, sb * SW : (sb + 1) * SW
                    ],
                )
                encN_t = encNp.tile([P, CH, H], BF16, name="encN_t")
                nc.sync.dma_start(
                    out=encN_t[:],
                    in_=encN_d.ap()[b].rearrange("(c p) h -> p c h", p=P)[
                        :, sb * CH : (sb + 1) * CH, :
                    ],
                )

                for c in range(CH):
                    q = sb * CH + c
                    # e_proj for 128 s-rows: [128, 1024] over two psum banks
                    ep_ps = ep_psum.tile([P, H], F32, name="ep_ps")
                    for kc in range(KC):
                        for h in range(2):
                            nc.tensor.matmul(
                                ep_ps[:, h * 512 : (h + 1) * 512],
                                encT_t[:, kc, c * P : (c + 1) * P],
                                uw_t[:, kc, h * 512 : (h + 1) * 512],
                                start=(kc == 0),
                                stop=(kc == KC - 1),
                            )

                    def emit_chain(q, ep_ps):
                        # + row bias, tanh, V-weighted reduce -> scores column
                        es_t = esp.tile([P, H], F32, name="es_t")
                        nc.vector.tensor_add(es_t[:], ep_ps[:], hb128_t[:])
                        tanh_t = tanhp.tile([P, H], BF16, name="tanh_t")
                        nc.scalar.activation(tanh_t[:], es_t[:], AF.Tanh)
                        scr_t = scrp.tile([P, H], BF16, name="scr_t")
                        nc.vector.tensor_mul(scr_t[:], tanh_t[:], v128_t[:])
                        acc_t = accp.tile([P, 1], F32, name="acc_t")
                        nc.vector.reduce_sum(
                            acc_t[:], scr_t[:], axis=mybir.AxisListType.X
                        )
                        # a-column = exp(scores + V_b), already in lhsT layout
                        nc.scalar.activation(
                            acol_t[:, q : q + 1], acc_t[:], AF.Exp, bias=vb_t[:]
                        )

                    emit_chain(q, ep_ps)
                    pending.append(make_ctx(q, encN_t, c))
                    drain_pending(keep=2)

            drain_pending(keep=0)  # batch boundary: finish ctx/den for b
            ctx_ps0, ctx_ps1, den_ps = cps["c0"], cps["c1"], cps["d"]

            den_t = outp.tile([1, 1], F32, name="den_t")
            rec_t = outp.tile([1, 1], F32, name="rec_t")
            nc.vector.tensor_copy(den_t[:], den_ps[:])
            nc.vector.reciprocal(rec_t[:], den_t[:])
            rec128_t = accp.tile([P, 1], F32, name="rec128_t")
            nc.gpsimd.partition_broadcast(rec128_t[:], rec_t[:], channels=P)

            acoln_t = outp.tile([P, NQ], F32, name="acoln_t")
            nc.vector.tensor_scalar_mul(acoln_t[:], acol_t[:], rec128_t[:])
            nc.sync.dma_start(out=attn_d.ap()[b], in_=acoln_t[:])

            cs_t = outp.tile([1, H], F32, name="cs_t")
            nc.vector.tensor_scalar_mul(cs_t[0:1, 0:512], ctx_ps0[:], rec_t[:])
            nc.vector.tensor_scalar_mul(cs_t[0:1, 512:1024], ctx_ps1[:], rec_t[:])
            nc.sync.dma_start(out=ctx_d.ap()[b : b + 1, :], in_=cs_t[:])

    nc.compile()
    return nc


def _prep_inputs(hidden, enc, W_w, W_b, U_w, U_b, V_w):
    bf16 = ml_dtypes.bfloat16
    uwT = np.ascontiguousarray(U_w.T).astype(bf16)
    wwT = np.ascontiguousarray(W_w.T).astype(bf16)
    wub4 = np.ascontiguousarray(
        np.broadcast_to((W_b + U_b)[None, :], (BL, H))
    ).astype(np.float32)
    v128 = np.ascontiguousarray(
        np.broadcast_to(V_w.reshape(1, H), (P, H))
    ).astype(bf16)

    in_maps = []
    for i in range(NCORES):
        sl = slice(i * BL, (i + 1) * BL)
        e = enc[sl]
        in_maps.append(
            {
                "encT": np.ascontiguousarray(e.transpose(0, 2, 1)).astype(bf16),
                "encN": e.astype(bf16),
                "uwT": uwT,
                "wwT": wwT,
                "hidT": np.ascontiguousarray(hidden[sl, 0, :].T).astype(bf16),
                "wub4": wub4,
                "v128": v128,
            }
        )
    return in_maps


def run(inputs: dict, trace: bool = False):
    """Build + run; returns ((context, attention), BassKernelResults)."""
    hidden = np.asarray(inputs["hidden"], dtype=np.float32)
    enc = np.asarray(inputs["encoder_output"], dtype=np.float32)
    W_w = np.asarray(inputs["W_w"], dtype=np.float32)
    W_b = np.asarray(inputs["W_b"], dtype=np.float32)
    U_w = np.asarray(inputs["U_w"], dtype=np.float32)
    U_b = np.asarray(inputs["U_b"], dtype=np.float32)
    V_w = np.asarray(inputs["V_w"], dtype=np.float32)
    V_b = np.asarray(inputs["V_b"], dtype=np.float32)

    nc = build_program(float(V_b.reshape(-1)[0]))
    in_maps = _prep_inputs(hidden, enc, W_w, W_b, U_w, U_b, V_w)
    try:
        res = run_bass_kernel_spmd(nc, in_maps, list(range(NCORES)), trace=trace)
    except Exception:
        # a previously wedged NeuronCore can fail the first execute; one
        # clean retry recovers it
        res = run_bass_kernel_spmd(nc, in_maps, list(range(NCORES)), trace=trace)

    ctx = np.concatenate(
        [np.asarray(res.results[i]["ctx_out"]) for i in range(NCORES)], axis=0
    ).astype(np.float32)[:, None, :]
    attn_col = np.concatenate(
        [np.asarray(res.results[i]["attn_out"]) for i in range(NCORES)], axis=0
    ).astype(np.float32)
    # attn_col[b, p, q] holds attention[b, q*128 + p]
    attn = attn_col.transpose(0, 2, 1).reshape(B, S)[:, None, :]
    return (ctx, attn), res


def kernel(**inputs) -> tuple:
    out, _ = run(inputs, trace=False)
    return out
